# revision 32
# baseline (speedup 1.0000x reference)
"""MultiHeadEMA on 8 Trainium2 NeuronCores — packed-phase layout.

Strategy
--------
Channel-sharded: embed_dim=1024 -> 8 slices of 128 channels, one per core.
The FFT conv is an order-2 IIR  y_n[l] = q_n y_n[l-1] + x[l],
out = silu(c0 y0 + c1 y1 + omega x), decimated by 4 for the DVE scan:
    Y_n[j] = y_n[4j+3] satisfies  Y_n[j] = q_n^4 Y_n[j-1] + u_n[j]
    u_n[j] = sum_{i=0..3} q_n^{3-i} x[4j+i]
    out[4j+r] = sum_n c_n q_n^{r+1} Y_n[j-1]
              + sum_{i<=r} wsum_{r-i} x[4j+i] + omega x[4j+r]
with wsum_s = sum_n c_n q_n^s.  (c_n is folded into the scan input.)

Layout trick: SBUF x tiles pack (phase i, channel) on partitions —
[4 phases x 32 channels = 128, J=1024].  All taps of u and ALL phases'
x-terms then live at the same j, so each is ONE block-sparse 128x128
matmul instead of 4-6 diagonal ones:
  - u:    Wu[(i,c),(n,c)] = c_n q_n^{3-i}    (64-col weight; two rhs tiles
          fill the [2 states x 64 ch] scan tile at partition offsets 0/64)
  - x-pyramid + omega residual: Wp[(i,c),(r,c)] = wsum_{r-i} + omega I
  - Y-terms: Wy[(n,c),(r,c)] = q_n^{r+1} on the j-1-shifted scan output
PE drops from 40 to 24 512-wide matmul units per batch; DVE runs only the
2 scans per batch; Scalar does Silu straight out of PSUM.

PSUM: 3 u-tiles (2 banks each) + 2 out-chunks (1 bank) = 8 banks exactly.
Schedule: u(b+1) is queued on PE before the out-chunks of batch b so the
PE never waits on the scans.  DMA: x prefetched whole on the two HWDGE
rings (batch 0 split per-quarter so the first matmul starts early),
weights + the last slab on the gpsimd SWDGE ring, stores interleave on
both HWDGE rings.
"""

import numpy as np
import ml_dtypes

import concourse.bass as bass
import concourse.bacc as bacc
import concourse.tile as tile
from concourse import mybir
from concourse.bass_utils import run_bass_kernel_spmd

SEQ_LEN, BSZ, EMBED_DIM, NDIM = 4096, 4, 1024, 2
N_CORES = 8
D_PER = EMBED_DIM // N_CORES  # 128 channels/core
SCALE = (1.0 / NDIM) ** 0.5
DEC = 4                   # decimation factor = phases per j-block
J = SEQ_LEN // DEC        # decimated length 1024
CH = 512                  # matmul chunk (one fp32 PSUM bank)
NQ = 4                    # channel quarters (32 ch each) per core
F32 = mybir.dt.float32
BF16 = mybir.dt.bfloat16
AF = mybir.ActivationFunctionType
ALU = mybir.AluOpType


def build_bass():
    nc = bacc.Bacc(name="multihead_ema")
    x = nc.dram_tensor("x", [BSZ, D_PER, NQ, J], BF16, kind="ExternalInput")
    wu = nc.dram_tensor("wu", [D_PER, NQ, 64], BF16, kind="ExternalInput")
    wp = nc.dram_tensor("wp", [D_PER, NQ, D_PER], BF16, kind="ExternalInput")
    wy = nc.dram_tensor("wy", [D_PER, NQ, D_PER], BF16, kind="ExternalInput")
    q4s = nc.dram_tensor("q4s", [D_PER, 2], F32, kind="ExternalInput")
    out = nc.dram_tensor("out", [BSZ, 2, D_PER, 2, J], BF16, kind="ExternalOutput")

    with tile.TileContext(nc) as tc:
        with (
            tc.tile_pool(name="const", bufs=1) as const,
            tc.tile_pool(name="xup", bufs=4) as xup,
            tc.tile_pool(name="yp", bufs=4) as yp,
            tc.tile_pool(name="obp", bufs=4) as obp,
            tc.tile_pool(name="psu", bufs=2, space="PSUM") as psu,
            tc.tile_pool(name="psc", bufs=2, space="PSUM") as psc,
        ):
            # --- PE clock warm-up: the tensor engine ramps 0.65 -> 2.4GHz
            # only after ~3us of continuous work, so a stream of dummy
            # matmuls on a memset tile spans the DMA wait and the real
            # matmuls start at full clock.
            zcon = const.tile([D_PER, CH], BF16)
            nc.gpsimd.memset(zcon[:, :], 0.0)
            Sd = psu.tile([D_PER, J], F32, tag="s", name="Sd")
            for _ in range(13):
                nc.tensor.matmul(Sd[:, 0:CH], zcon[:, 0:D_PER], zcon[:, :],
                                 start=True, stop=True)

            # --- DMA: the gating weights ride the fast HWDGE rings first;
            # bulk weights go on the SWDGE ring; x lands quarter-by-quarter
            # in consumption order, alternating the two HWDGE rings.
            wusb = const.tile([D_PER, NQ, 64], BF16)
            nc.sync.dma_start(out=wusb[:, :, :], in_=wu[:, :, :])
            q4sb = const.tile([D_PER, 2], F32)
            nc.scalar.dma_start(out=q4sb[:, :], in_=q4s[:, :])

            wpsb = const.tile([D_PER, NQ, D_PER], BF16)
            wysb = const.tile([D_PER, NQ, D_PER], BF16)

            # DMA schedule, tuned to two facts from the traces: (a) each
            # dma_start costs ~650ns of serial per-ring descriptor
            # generation, (b) the 16 hw queues drain all enqueued traffic
            # ~FIFO, so a transfer completes late by everything queued before
            # it.  So: ~256KB pieces, enqueued strictly in consumption
            # order.  Batch 0 is split column-wise across both rings so the
            # first matmul chunks fire earliest; the scalar(=Activation)
            # ring stays light because silus need that queue from ~15us on.
            xts = [xup.tile([D_PER, NQ, J], BF16, tag="xt", name=f"xt{b}")
                   for b in range(BSZ)]
            for qd in range(NQ):
                eng = nc.sync if qd % 2 == 0 else nc.scalar
                eng.dma_start(out=xts[0][:, qd, :], in_=x[0, :, qd, :])
            # strict need-order: all of batch 1, then the bulk weights
            # (first pyramid matmul wants them at ~17us)
            nc.sync.dma_start(out=xts[1][:, 0, :], in_=x[1, :, 0, :])
            nc.scalar.dma_start(out=xts[1][:, 1, :], in_=x[1, :, 1, :])
            nc.sync.dma_start(out=xts[1][:, 2, :], in_=x[1, :, 2, :])
            nc.scalar.dma_start(out=xts[1][:, 3, :], in_=x[1, :, 3, :])
            nc.sync.dma_start(out=wpsb[:, :, :], in_=wp[:, :, :])
            nc.scalar.dma_start(out=wysb[:, :, :], in_=wy[:, :, :])
            for b in (2, 3):
                for qd in range(NQ):
                    # sync-heavy: the scalar ring's queue must be free for
                    # the silus by the time batch-0 outputs are ready
                    eng = nc.scalar if (b == 2 and qd == 0) else nc.sync
                    eng.dma_start(out=xts[b][:, qd, :], in_=x[b, :, qd, :])
            # dummy 1-col silu: hoists the lazy Silu act-table load (~1.3us)
            # into the idle startup window instead of mid-kernel
            warm = const.tile([D_PER, 1], BF16)
            nc.scalar.activation(out=warm[:, :], in_=q4sb[:, 0:1], func=AF.Silu)

            q4b = [q4sb[:, h : h + 1].to_broadcast([D_PER, J]) for h in range(2)]

            def emit_u(b):
                """u matmuls into PSUM (partition-packed) + DVE scans."""
                xt = xts[b]
                Ys = []
                for h in range(2):
                    S = psu.tile([D_PER, J], F32, tag="s")
                    for qq in (2 * h, 2 * h + 1):
                        pbase = (qq % 2) * 64
                        for g in range(2):
                            s = bass.ts(g, CH)
                            nc.tensor.matmul(
                                S[pbase : pbase + 64, s], wusb[:, qq, :],
                                xt[:, qq, s], start=True, stop=True)
                    # scan in 512-col chunks chained via `initial`: the
                    # first Y-term matmuls fire ~1.1us earlier
                    Y = yp.tile([D_PER, 1 + J], BF16, tag="y")
                    nc.vector.tensor_tensor_scan(
                        out=Y[:, 1 : 1 + CH], data0=q4b[h][:, 0:CH],
                        data1=S[:, 0:CH], initial=0.0,
                        op0=ALU.mult, op1=ALU.add)
                    nc.vector.tensor_tensor_scan(
                        out=Y[:, 1 + CH : 1 + J], data0=q4b[h][:, 0:CH],
                        data1=S[:, CH:J], initial=Y[:, CH : CH + 1],
                        op0=ALU.mult, op1=ALU.add)
                    Ys.append(Y)
                return Ys

            def emit_out(b, Ys):
                xt = xts[b]
                for h in range(2):
                    Y = Ys[h]
                    ob = obp.tile([D_PER, 2, J], BF16, tag="ob")
                    Ps = [psc.tile([D_PER, J], F32, tag="p", name=f"P{qq}")
                          for qq in range(2)]
                    # both pyramids ahead of the Y-terms: hides Wy-weight
                    # and scan latency behind ~1.8us of x-only matmuls
                    for qq in range(2):
                        q = 2 * h + qq
                        nc.tensor.matmul(Ps[qq][:, 0:CH], wpsb[:, q, :],
                                         xt[:, q, 0:CH], start=True, stop=False)
                        nc.tensor.matmul(Ps[qq][:, CH:J], wpsb[:, q, :],
                                         xt[:, q, CH:J], start=True, stop=False)
                    last = b == BSZ - 1 and h == 1
                    for qq in range(2):
                        q = 2 * h + qq
                        # Y[j-1] terms: scan output lives at Y[:, 1+j]; col 0
                        # of chunk 0 (j=0, Y[-1]=0) keeps its pyramid value
                        nc.tensor.matmul(Ps[qq][:, 1:CH], wysb[:, q, :],
                                         Y[:, 1:CH], start=False, stop=True)
                        if last and qq == 1:
                            # final quarter: chunked silu so the drain
                            # starts before the very last matmul
                            nc.scalar.activation(out=ob[:, 1, 0:CH],
                                                 in_=Ps[1][:, 0:CH],
                                                 func=AF.Silu)
                        nc.tensor.matmul(Ps[qq][:, CH:J], wysb[:, q, :],
                                         Y[:, CH:J], start=False, stop=True)
                        if last and qq == 1:
                            nc.scalar.activation(out=ob[:, 1, CH:J],
                                                 in_=Ps[1][:, CH:J],
                                                 func=AF.Silu)
                        else:
                            nc.scalar.activation(out=ob[:, qq, :],
                                                 in_=Ps[qq][:, :],
                                                 func=AF.Silu)
                    # one 512KB store per half-batch keeps the ring-sequencer
                    # descriptor-generation cost down; none on the scalar
                    # ring -- its queue belongs to the silus.  The very last
                    # store splits across two rings to halve the drain tail.
                    if last:
                        nc.sync.dma_start(out=out[b, h, :, 0:1, :],
                                          in_=ob[:, 0:1, :])
                        nc.gpsimd.dma_start(out=out[b, h, :, 1, 0:CH],
                                            in_=ob[:, 1, 0:CH])
                        nc.sync.dma_start(out=out[b, h, :, 1, CH:J],
                                          in_=ob[:, 1, CH:J])
                    else:
                        eng = nc.gpsimd if h == 0 else nc.sync
                        eng.dma_start(out=out[b, h, :, :, :], in_=ob[:, :, :])

            # software pipeline: u(b+1) rides ahead of out-chunks(b)
            pend = emit_u(0)
            for b in range(BSZ):
                nxt = emit_u(b + 1) if b + 1 < BSZ else None
                emit_out(b, pend)
                pend = nxt

    nc.compile()
    return nc


_CACHE: dict = {}


def _get_nc():
    if "nc" not in _CACHE:
        _CACHE["nc"] = build_bass()
    return _CACHE["nc"]


def make_in_maps(inputs):
    x = np.asarray(inputs["x"], np.float32)
    delta = np.asarray(inputs["delta"], np.float64).reshape(EMBED_DIM, NDIM)
    alpha = np.asarray(inputs["alpha"], np.float64).reshape(EMBED_DIM, NDIM)
    beta = np.asarray(inputs["beta"], np.float64).reshape(EMBED_DIM, NDIM)
    gamma = np.asarray(inputs["gamma"], np.float64).reshape(EMBED_DIM, NDIM)
    omega = np.asarray(inputs["omega"], np.float64)  # [D]

    p = 1.0 / (1.0 + np.exp(-delta))
    q = 1.0 - p / (1.0 + np.exp(-alpha))              # [D, 2]
    c = p * beta * gamma * SCALE                       # [D, 2]
    wsum = np.stack([(c * q**s).sum(1) for s in range(DEC)], 1)  # [D, 4]

    # x: [L, B, D] -> per core [B, 128(p=i*32+cc), Q, J]
    xr = x.reshape(J, DEC, BSZ, N_CORES, NQ, 32)       # [j, i, b, core, q, cc]
    xp = xr.transpose(3, 2, 1, 5, 4, 0)                # [core, b, i, cc, q, j]
    xp = np.ascontiguousarray(
        xp.reshape(N_CORES, BSZ, D_PER, NQ, J)).astype(ml_dtypes.bfloat16)

    cc = np.arange(32)
    in_maps = []
    for core in range(N_CORES):
        wuc = np.zeros((D_PER, NQ, 64), np.float64)
        wpc = np.zeros((D_PER, NQ, D_PER), np.float64)
        wyc = np.zeros((D_PER, NQ, D_PER), np.float64)
        q4c = np.zeros((D_PER, 2), np.float64)
        for qd in range(NQ):
            ch = core * D_PER + qd * 32 + cc           # [32]
            for i in range(DEC):
                for n in range(NDIM):
                    wuc[i * 32 + cc, qd, n * 32 + cc] = c[ch, n] * q[ch, n] ** (3 - i)
                for r in range(i, DEC):
                    wpc[i * 32 + cc, qd, r * 32 + cc] = wsum[ch, r - i] + (
                        omega[ch] if r == i else 0.0)
            # scan-tile partition layout (set by the u matmul placement):
            # p = (quarter parity)*64 + state*32 + cc
            for n in range(NDIM):
                for r in range(DEC):
                    wyc[(qd % 2) * 64 + n * 32 + cc, qd, r * 32 + cc] = (
                        q[ch, n] ** (r + 1))
        for h in range(2):
            for e in range(2):
                for n in range(NDIM):
                    q4c[e * 64 + n * 32 + cc, h] = (
                        q[core * D_PER + h * 64 + e * 32 + cc, n] ** 4)
        in_maps.append({
            "x": np.ascontiguousarray(xp[core]),
            "wu": wuc.astype(ml_dtypes.bfloat16),
            "wp": wpc.astype(ml_dtypes.bfloat16),
            "wy": wyc.astype(ml_dtypes.bfloat16),
            "q4s": q4c.astype(np.float32),
        })
    return in_maps


def gather_out(results):
    out = np.empty((SEQ_LEN, BSZ, EMBED_DIM), np.float32)
    for core in range(N_CORES):
        # [B, half, 128(p=r*32+cc), qq, J] -> [l=4j+r, b, ch=(2h+qq)*32+cc]
        arr = results[core]["out"].astype(np.float32)
        a = arr.reshape(BSZ, 2, DEC, 32, 2, J).transpose(5, 2, 0, 1, 4, 3)
        out[:, :, core * D_PER : (core + 1) * D_PER] = a.reshape(
            SEQ_LEN, BSZ, D_PER)
    return out


def _run(inputs, **kwargs):
    nc = _get_nc()
    in_maps = make_in_maps(inputs)
    res = run_bass_kernel_spmd(nc, in_maps, core_ids=list(range(N_CORES)), **kwargs)
    return gather_out(res.results), res


def kernel(**inputs) -> np.ndarray:
    out, _ = _run(inputs)
    return out


# revision 33
# speedup vs baseline: 1.0596x; 1.0596x over previous
"""MultiHeadEMA on 8 Trainium2 NeuronCores — packed-phase layout.

Strategy
--------
Channel-sharded: embed_dim=1024 -> 8 slices of 128 channels, one per core.
The FFT conv is an order-2 IIR  y_n[l] = q_n y_n[l-1] + x[l],
out = silu(c0 y0 + c1 y1 + omega x), decimated by 4 for the DVE scan:
    Y_n[j] = y_n[4j+3] satisfies  Y_n[j] = q_n^4 Y_n[j-1] + u_n[j]
    u_n[j] = sum_{i=0..3} q_n^{3-i} x[4j+i]
    out[4j+r] = sum_n c_n q_n^{r+1} Y_n[j-1]
              + sum_{i<=r} wsum_{r-i} x[4j+i] + omega x[4j+r]
with wsum_s = sum_n c_n q_n^s.  (c_n is folded into the scan input.)

Layout trick: SBUF x tiles pack (phase i, channel) on partitions —
[4 phases x 32 channels = 128, J=1024].  All taps of u and ALL phases'
x-terms then live at the same j, so each is ONE block-sparse 128x128
matmul instead of 4-6 diagonal ones:
  - u:    Wu[(i,c),(n,c)] = c_n q_n^{3-i}    (64-col weight; two rhs tiles
          fill the [2 states x 64 ch] scan tile at partition offsets 0/64)
  - x-pyramid + omega residual: Wp[(i,c),(r,c)] = wsum_{r-i} + omega I
  - Y-terms: Wy[(n,c),(r,c)] = q_n^{r+1} on the j-1-shifted scan output
PE drops from 40 to 24 512-wide matmul units per batch; DVE runs only the
2 scans per batch; Scalar does Silu straight out of PSUM.

PSUM: 3 u-tiles (2 banks each) + 2 out-chunks (1 bank) = 8 banks exactly.
Schedule: u(b+1) is queued on PE before the out-chunks of batch b so the
PE never waits on the scans.  DMA: x prefetched whole on the two HWDGE
rings (batch 0 split per-quarter so the first matmul starts early),
weights + the last slab on the gpsimd SWDGE ring, stores interleave on
both HWDGE rings.
"""

import numpy as np
import ml_dtypes

import concourse.bass as bass
import concourse.bacc as bacc
import concourse.tile as tile
from concourse import mybir
from concourse.bass_utils import run_bass_kernel_spmd

SEQ_LEN, BSZ, EMBED_DIM, NDIM = 4096, 4, 1024, 2
N_CORES = 8
D_PER = EMBED_DIM // N_CORES  # 128 channels/core
SCALE = (1.0 / NDIM) ** 0.5
DEC = 4                   # decimation factor = phases per j-block
J = SEQ_LEN // DEC        # decimated length 1024
CH = 512                  # matmul chunk (one fp32 PSUM bank)
NQ = 4                    # channel quarters (32 ch each) per core
F32 = mybir.dt.float32
BF16 = mybir.dt.bfloat16
AF = mybir.ActivationFunctionType
ALU = mybir.AluOpType


def build_bass():
    nc = bacc.Bacc(name="multihead_ema")
    x = nc.dram_tensor("x", [BSZ, D_PER, NQ, J], BF16, kind="ExternalInput")
    wu = nc.dram_tensor("wu", [D_PER, NQ, 64], BF16, kind="ExternalInput")
    wp = nc.dram_tensor("wp", [D_PER, NQ, D_PER], BF16, kind="ExternalInput")
    wy = nc.dram_tensor("wy", [D_PER, NQ, D_PER], BF16, kind="ExternalInput")
    q4s = nc.dram_tensor("q4s", [D_PER, 2], F32, kind="ExternalInput")
    out = nc.dram_tensor("out", [BSZ, 2, D_PER, 2, J], BF16, kind="ExternalOutput")

    with tile.TileContext(nc) as tc:
        with (
            tc.tile_pool(name="const", bufs=1) as const,
            tc.tile_pool(name="xup", bufs=4) as xup,
            tc.tile_pool(name="yp", bufs=4) as yp,
            tc.tile_pool(name="obp", bufs=4) as obp,
            tc.tile_pool(name="psu", bufs=2, space="PSUM") as psu,
            tc.tile_pool(name="psc", bufs=2, space="PSUM") as psc,
        ):
            # --- PE clock warm-up: the tensor engine ramps 0.65 -> 2.4GHz
            # only after ~3us of continuous work, so a stream of dummy
            # matmuls on a memset tile spans the DMA wait and the real
            # matmuls start at full clock.
            zcon = const.tile([D_PER, CH], BF16)
            nc.gpsimd.memset(zcon[:, :], 0.0)
            Sd = psu.tile([D_PER, J], F32, tag="s", name="Sd")
            for _ in range(13):
                nc.tensor.matmul(Sd[:, 0:CH], zcon[:, 0:D_PER], zcon[:, :],
                                 start=True, stop=True)

            # --- DMA: the gating weights ride the fast HWDGE rings first;
            # bulk weights go on the SWDGE ring; x lands quarter-by-quarter
            # in consumption order, alternating the two HWDGE rings.
            wusb = const.tile([D_PER, NQ, 64], BF16)
            nc.sync.dma_start(out=wusb[:, :, :], in_=wu[:, :, :])
            q4sb = const.tile([D_PER, 2], F32)
            nc.scalar.dma_start(out=q4sb[:, :], in_=q4s[:, :])

            wpsb = const.tile([D_PER, NQ, D_PER], BF16)
            wysb = const.tile([D_PER, NQ, D_PER], BF16)

            # DMA schedule, tuned to two facts from the traces: (a) each
            # dma_start costs ~650ns of serial per-ring descriptor
            # generation, (b) the 16 hw queues drain all enqueued traffic
            # ~FIFO, so a transfer completes late by everything queued before
            # it.  So: ~256KB pieces, enqueued strictly in consumption
            # order.  Batch 0 is split column-wise across both rings so the
            # first matmul chunks fire earliest; the scalar(=Activation)
            # ring stays light because silus need that queue from ~15us on.
            xts = [xup.tile([D_PER, NQ, J], BF16, tag="xt", name=f"xt{b}")
                   for b in range(BSZ)]
            for qd in range(NQ):
                eng = nc.sync if qd % 2 == 0 else nc.scalar
                eng.dma_start(out=xts[0][:, qd, :], in_=x[0, :, qd, :])
            # bulk weights slot mid-batch-1: late enough to keep the early
            # FIFO clear for batch 0, early enough to beat the ring-credit
            # stall that hits around the 6th dma_start per ring
            nc.sync.dma_start(out=xts[1][:, 0, :], in_=x[1, :, 0, :])
            nc.scalar.dma_start(out=xts[1][:, 1, :], in_=x[1, :, 1, :])
            nc.sync.dma_start(out=wpsb[:, :, :], in_=wp[:, :, :])
            nc.scalar.dma_start(out=wysb[:, :, :], in_=wy[:, :, :])
            nc.sync.dma_start(out=xts[1][:, 2, :], in_=x[1, :, 2, :])
            nc.scalar.dma_start(out=xts[1][:, 3, :], in_=x[1, :, 3, :])
            for b in (2, 3):
                for qd in range(NQ):
                    # sync-heavy: the scalar ring's queue must be free for
                    # the silus by the time batch-0 outputs are ready
                    eng = nc.scalar if (b == 2 and qd == 0) else nc.sync
                    eng.dma_start(out=xts[b][:, qd, :], in_=x[b, :, qd, :])
            # dummy 1-col silu: hoists the lazy Silu act-table load (~1.3us)
            # into the idle startup window instead of mid-kernel
            warm = const.tile([D_PER, 1], BF16)
            nc.scalar.activation(out=warm[:, :], in_=q4sb[:, 0:1], func=AF.Silu)

            q4b = [q4sb[:, h : h + 1].to_broadcast([D_PER, J]) for h in range(2)]

            def emit_u(b):
                """u matmuls into PSUM (partition-packed) + DVE scans."""
                xt = xts[b]
                Ys = []
                for h in range(2):
                    S = psu.tile([D_PER, J], F32, tag="s")
                    for qq in (2 * h, 2 * h + 1):
                        pbase = (qq % 2) * 64
                        for g in range(2):
                            s = bass.ts(g, CH)
                            nc.tensor.matmul(
                                S[pbase : pbase + 64, s], wusb[:, qq, :],
                                xt[:, qq, s], start=True, stop=True)
                    # scan in 512-col chunks chained via `initial`: the
                    # first Y-term matmuls fire ~1.1us earlier
                    Y = yp.tile([D_PER, 1 + J], BF16, tag="y")
                    nc.vector.tensor_tensor_scan(
                        out=Y[:, 1 : 1 + CH], data0=q4b[h][:, 0:CH],
                        data1=S[:, 0:CH], initial=0.0,
                        op0=ALU.mult, op1=ALU.add)
                    nc.vector.tensor_tensor_scan(
                        out=Y[:, 1 + CH : 1 + J], data0=q4b[h][:, 0:CH],
                        data1=S[:, CH:J], initial=Y[:, CH : CH + 1],
                        op0=ALU.mult, op1=ALU.add)
                    Ys.append(Y)
                return Ys

            def emit_out(b, Ys):
                xt = xts[b]
                for h in range(2):
                    Y = Ys[h]
                    ob = obp.tile([D_PER, 2, J], BF16, tag="ob")
                    Ps = [psc.tile([D_PER, J], F32, tag="p", name=f"P{qq}")
                          for qq in range(2)]
                    # both pyramids ahead of the Y-terms: hides Wy-weight
                    # and scan latency behind ~1.8us of x-only matmuls
                    for qq in range(2):
                        q = 2 * h + qq
                        nc.tensor.matmul(Ps[qq][:, 0:CH], wpsb[:, q, :],
                                         xt[:, q, 0:CH], start=True, stop=False)
                        nc.tensor.matmul(Ps[qq][:, CH:J], wpsb[:, q, :],
                                         xt[:, q, CH:J], start=True, stop=False)
                    last = b == BSZ - 1 and h == 1
                    for qq in range(2):
                        q = 2 * h + qq
                        # Y[j-1] terms: scan output lives at Y[:, 1+j]; col 0
                        # of chunk 0 (j=0, Y[-1]=0) keeps its pyramid value
                        nc.tensor.matmul(Ps[qq][:, 1:CH], wysb[:, q, :],
                                         Y[:, 1:CH], start=False, stop=True)
                        if last and qq == 1:
                            # final quarter: chunked silu so the drain
                            # starts before the very last matmul
                            nc.scalar.activation(out=ob[:, 1, 0:CH],
                                                 in_=Ps[1][:, 0:CH],
                                                 func=AF.Silu)
                        nc.tensor.matmul(Ps[qq][:, CH:J], wysb[:, q, :],
                                         Y[:, CH:J], start=False, stop=True)
                        if last and qq == 1:
                            nc.scalar.activation(out=ob[:, 1, CH:J],
                                                 in_=Ps[1][:, CH:J],
                                                 func=AF.Silu)
                        else:
                            nc.scalar.activation(out=ob[:, qq, :],
                                                 in_=Ps[qq][:, :],
                                                 func=AF.Silu)
                    # one 512KB store per half-batch keeps the ring-sequencer
                    # descriptor-generation cost down; none on the scalar
                    # ring -- its queue belongs to the silus.  The very last
                    # store splits across two rings to halve the drain tail.
                    if last:
                        nc.sync.dma_start(out=out[b, h, :, 0:1, :],
                                          in_=ob[:, 0:1, :])
                        nc.gpsimd.dma_start(out=out[b, h, :, 1, 0:CH],
                                            in_=ob[:, 1, 0:CH])
                        nc.sync.dma_start(out=out[b, h, :, 1, CH:J],
                                          in_=ob[:, 1, CH:J])
                    else:
                        eng = nc.gpsimd if h == 0 else nc.sync
                        eng.dma_start(out=out[b, h, :, :, :], in_=ob[:, :, :])

            # software pipeline: u(b+1) rides ahead of out-chunks(b)
            pend = emit_u(0)
            for b in range(BSZ):
                nxt = emit_u(b + 1) if b + 1 < BSZ else None
                emit_out(b, pend)
                pend = nxt

    nc.compile()
    return nc


_CACHE: dict = {}


def _get_nc():
    if "nc" not in _CACHE:
        _CACHE["nc"] = build_bass()
    return _CACHE["nc"]


def make_in_maps(inputs):
    x = np.asarray(inputs["x"], np.float32)
    delta = np.asarray(inputs["delta"], np.float64).reshape(EMBED_DIM, NDIM)
    alpha = np.asarray(inputs["alpha"], np.float64).reshape(EMBED_DIM, NDIM)
    beta = np.asarray(inputs["beta"], np.float64).reshape(EMBED_DIM, NDIM)
    gamma = np.asarray(inputs["gamma"], np.float64).reshape(EMBED_DIM, NDIM)
    omega = np.asarray(inputs["omega"], np.float64)  # [D]

    p = 1.0 / (1.0 + np.exp(-delta))
    q = 1.0 - p / (1.0 + np.exp(-alpha))              # [D, 2]
    c = p * beta * gamma * SCALE                       # [D, 2]
    wsum = np.stack([(c * q**s).sum(1) for s in range(DEC)], 1)  # [D, 4]

    # x: [L, B, D] -> per core [B, 128(p=i*32+cc), Q, J]
    xr = x.reshape(J, DEC, BSZ, N_CORES, NQ, 32)       # [j, i, b, core, q, cc]
    xp = xr.transpose(3, 2, 1, 5, 4, 0)                # [core, b, i, cc, q, j]
    xp = np.ascontiguousarray(
        xp.reshape(N_CORES, BSZ, D_PER, NQ, J)).astype(ml_dtypes.bfloat16)

    cc = np.arange(32)
    in_maps = []
    for core in range(N_CORES):
        wuc = np.zeros((D_PER, NQ, 64), np.float64)
        wpc = np.zeros((D_PER, NQ, D_PER), np.float64)
        wyc = np.zeros((D_PER, NQ, D_PER), np.float64)
        q4c = np.zeros((D_PER, 2), np.float64)
        for qd in range(NQ):
            ch = core * D_PER + qd * 32 + cc           # [32]
            for i in range(DEC):
                for n in range(NDIM):
                    wuc[i * 32 + cc, qd, n * 32 + cc] = c[ch, n] * q[ch, n] ** (3 - i)
                for r in range(i, DEC):
                    wpc[i * 32 + cc, qd, r * 32 + cc] = wsum[ch, r - i] + (
                        omega[ch] if r == i else 0.0)
            # scan-tile partition layout (set by the u matmul placement):
            # p = (quarter parity)*64 + state*32 + cc
            for n in range(NDIM):
                for r in range(DEC):
                    wyc[(qd % 2) * 64 + n * 32 + cc, qd, r * 32 + cc] = (
                        q[ch, n] ** (r + 1))
        for h in range(2):
            for e in range(2):
                for n in range(NDIM):
                    q4c[e * 64 + n * 32 + cc, h] = (
                        q[core * D_PER + h * 64 + e * 32 + cc, n] ** 4)
        in_maps.append({
            "x": np.ascontiguousarray(xp[core]),
            "wu": wuc.astype(ml_dtypes.bfloat16),
            "wp": wpc.astype(ml_dtypes.bfloat16),
            "wy": wyc.astype(ml_dtypes.bfloat16),
            "q4s": q4c.astype(np.float32),
        })
    return in_maps


def gather_out(results):
    out = np.empty((SEQ_LEN, BSZ, EMBED_DIM), np.float32)
    for core in range(N_CORES):
        # [B, half, 128(p=r*32+cc), qq, J] -> [l=4j+r, b, ch=(2h+qq)*32+cc]
        arr = results[core]["out"].astype(np.float32)
        a = arr.reshape(BSZ, 2, DEC, 32, 2, J).transpose(5, 2, 0, 1, 4, 3)
        out[:, :, core * D_PER : (core + 1) * D_PER] = a.reshape(
            SEQ_LEN, BSZ, D_PER)
    return out


def _run(inputs, **kwargs):
    nc = _get_nc()
    in_maps = make_in_maps(inputs)
    res = run_bass_kernel_spmd(nc, in_maps, core_ids=list(range(N_CORES)), **kwargs)
    return gather_out(res.results), res


def kernel(**inputs) -> np.ndarray:
    out, _ = _run(inputs)
    return out
